# revision 24
# baseline (speedup 1.0000x reference)
"""PoissonGaussianReadout forward on 8 trn2 NeuronCores.

Math (eval mode): each neuron n samples feat[b] (a [36,36,1024] image per
batch, 1024 = C*T channels) bilinearly at a fixed point mu[n], then takes a
per-neuron dot with W[n,:], adds b[n], applies elu(y)+1.

Strategy:
  - Batch-shard: 8 cores x 2 batches each; every core computes all 4096
    neurons for its 2 batches (min HBM traffic: 10.6MB x-shard + 16.8MB W).
  - Sort neurons by their bilinear base cell p00 = y0*36+x0.  A block of
    <=128 sorted neurons spans a contiguous window of flat positions
    [pfirst, pfirst+WIN).  One fp32 matmul per (block, d-chunk):
       psum[n, (b,j)] += Wblk[d, n]^T @ feat[b, pfirst+j, d-chunk]
    i.e. the moving operand is a *contiguous slice* of feat -- no gather.
  - Each neuron's 4 bilinear corners live at window offsets
    (p00-pfirst)+{0,1,36,37}; a host-built sparse mask [n, b, j] holds the
    bilinear weights there.  DVE: masked = psum * mask; reduce_j -> z[n, b].
  - Bias + elu(y)+1 = exp(-relu(-y)) + relu(y) on device, output assembled
    and un-permuted on host.

The block structure / masks depend on mu, which is known when kernel() is
called; the Bass program is traced fresh per call, so they are baked in as
compile-time constants (correct for any input values).
"""
import sys
sys.path.insert(0, "/opt/trn_rl_repo")

import numpy as np

from concourse import bass, mybir, tile
from concourse.bass_utils import run_bass_kernel_spmd
from concourse.vector_clock import ScopedClock
import bass_rust

# problem constants
B, C, T, HH, WW = 16, 64, 16, 36, 36
N, D = 4096, C * T            # 4096 neurons, 1024 input dim
P = HH * WW                   # 1296 flat positions
NCHUNK = 8                    # D / 128 contraction chunks
NCORES = 8
BPC = B // NCORES             # batches per core = 2
WINMAX = 256                  # psum bank: 2*WIN <= 512 fp32
PAD = 38                      # max corner offset (37) + 1
FEATW = P + PAD               # padded feat width per (chunk, batch)

F32 = mybir.dt.float32

# dtype for x / W (the matmul operands).  bf16 halves DMA traffic and
# roughly halves PE time (FWL weight loads + 1cyc/col stream); PSUM
# accumulation stays fp32.  Set to False for bit-accurate fp32.
USE_BF16 = True
if USE_BF16:
    import ml_dtypes
    XW_DT = mybir.dt.bfloat16
    XW_NP = ml_dtypes.bfloat16
else:
    XW_DT = F32
    XW_NP = np.float32


def _split_waits(nc, max_waits=1):
    """Walrus in this image allows only ONE sem wait per instruction.
    Hoist extra waits onto injected same-engine NoOps placed immediately
    before the owning instruction (same engine + program order => same
    semantics)."""
    k = 0
    for fn in nc.m.functions:
        for blk in fn.blocks:
            insts = blk.instructions
            out = []
            for inst in insts:
                si = inst.sync_info
                if si is not None and si.on_wait and len(si.on_wait) > max_waits:
                    waits = list(si.on_wait)
                    for w in waits[:-max_waits]:
                        nop = mybir.InstNoOp(name=f"I-wsplit-{k}", ins=[], outs=[])
                        k += 1
                        nop.engine = inst.engine
                        nop.sync_info = bass_rust.SyncInfo(
                            on_wait=[w], on_update=[]
                        )
                        out.append(nop)
                    si.on_wait = waits[-max_waits:]
                    inst.sync_info = si
                out.append(inst)
            if len(out) != len(insts):
                insts.clear()
                insts.extend(out)


def _bilinear_tables(mu):
    """Per-neuron base cell p00, corner offsets (4) in {0,1,36,37}, corner
    weights (4), replicating reference float32 arithmetic exactly."""
    one, half = np.float32(1.0), np.float32(0.5)
    g = np.clip(mu.astype(np.float32), -one, one)
    ix = (g[:, 0] + one) * np.float32(WW * 0.5) - half
    iy = (g[:, 1] + one) * np.float32(HH * 0.5) - half
    x0 = np.floor(ix)
    y0 = np.floor(iy)
    wx1 = ix - x0
    wx0 = one - wx1
    wy1 = iy - y0
    wy0 = one - wy1

    xs = [x0, x0 + one]
    ys = [y0, y0 + one]
    wxs = [wx0, wx1]
    wys = [wy0, wy1]

    x0c = np.clip(x0, 0, WW - 1).astype(np.int64)
    y0c = np.clip(y0, 0, HH - 1).astype(np.int64)
    p00 = y0c * WW + x0c

    offs = np.zeros((4, N), np.int64)
    wgts = np.zeros((4, N), np.float32)
    k = 0
    for a in range(2):          # y corner
        for bb in range(2):     # x corner
            xx, yy = xs[bb], ys[a]
            valid = (xx >= 0) & (xx <= WW - 1) & (yy >= 0) & (yy <= HH - 1)
            xi = np.clip(xx, 0, WW - 1).astype(np.int64)
            yi = np.clip(yy, 0, HH - 1).astype(np.int64)
            offs[k] = yi * WW + xi - p00
            wgts[k] = (wys[a] * wxs[bb]) * valid.astype(np.float32)
            k += 1
    assert offs.min() >= 0 and offs.max() <= 37
    return p00, offs, wgts


def _make_blocks(p00_sorted):
    """Greedy blocks of <=128 sorted neurons with window <= WINMAX."""
    blocks = []  # (start, end) into sorted order
    s = 0
    n = len(p00_sorted)
    while s < n:
        pfirst = p00_sorted[s]
        e = s
        while e < n and e - s < 128 and (p00_sorted[e] - pfirst) + PAD <= WINMAX:
            e += 1
        blocks.append((s, e))
        s = e
    return blocks


def kernel(x, mu, sigma, W, b):
    x = np.ascontiguousarray(x, dtype=np.float32)
    W = np.ascontiguousarray(W, dtype=np.float32)
    b = np.asarray(b, dtype=np.float32)

    p00, offs, wgts = _bilinear_tables(mu)
    order = np.argsort(p00, kind="stable")
    p00s = p00[order]
    blocks = _make_blocks(p00s)
    nblk = len(blocks)

    # per-block host data
    wins, pfirsts, ms = [], [], []
    wparts, mparts = [], []
    biasT = np.zeros((128, 2 * nblk), np.float32)
    for i, (s, e) in enumerate(blocks):
        idx = order[s:e]
        m = e - s
        pfirst = int(p00s[s])
        win = int(p00s[e - 1]) - pfirst + PAD
        ms.append(m)
        pfirsts.append(pfirst)
        wins.append(win)
        # weights: [m,1024] -> [1024,m] -> [8,128,m] -> [128,8,m]
        blkW = W[idx, :].T.reshape(NCHUNK, 128, m).transpose(1, 0, 2)
        wparts.append(np.ascontiguousarray(blkW).reshape(128, NCHUNK * m))
        # mask [128, win] (b-dim broadcast on device)
        mk = np.zeros((128, win), np.float32)
        rel = (p00[idx] - pfirst)  # [m]
        for k in range(4):
            np.add.at(mk[:m], (np.arange(m), rel + offs[k][idx]), wgts[k][idx])
        mparts.append(mk)
        biasT[:m, 2 * i] = b[idx]
        biasT[:m, 2 * i + 1] = b[idx]

    # W packed per GRP-block group, partition-major: each group's DMA moves
    # [128, sum_i 8*m_i] with one long contiguous row per partition.
    GRP = 8
    ngrp = (nblk + GRP - 1) // GRP
    wgrps = []       # per-group [128, gcols] array
    gcol_off = []    # per-block column offset within its group (elements)
    for g in range(ngrp):
        gparts = []
        coff = 0
        for i in range(g * GRP, min(nblk, (g + 1) * GRP)):
            gcol_off.append(coff)
            gparts.append(wparts[i])          # [128, 8*m_i]
            coff += NCHUNK * ms[i]
        wgrps.append(np.ascontiguousarray(np.concatenate(gparts, axis=1)))
    wall = np.ascontiguousarray(np.concatenate(wgrps, axis=1))
    gw_off = np.cumsum([0] + [g.shape[1] for g in wgrps])  # col offset per group
    # all masks packed into one resident [128, sum(win)] tile
    mask_all = np.ascontiguousarray(np.concatenate(mparts, axis=1))
    moffs = np.cumsum([0] + [w for w in wins])
    mtot = int(mask_all.shape[1])

    # ---- build the Bass program (same for all cores) ----
    nc = bass.Bass()
    xs_h = nc.declare_dram_parameter("xs", [128, NCHUNK, BPC, FEATW], XW_DT,
                                     isOutput=False)
    wf_h = nc.declare_dram_parameter("wf", [128, int(wall.shape[1])], XW_DT,
                                     isOutput=False)
    mf_h = nc.declare_dram_parameter("mf", [128, mtot], F32, isOutput=False)
    bt_h = nc.declare_dram_parameter("bt", [128, 2 * nblk], F32, isOutput=False)
    z_h = nc.declare_dram_parameter("z", [128, 2 * nblk], F32, isOutput=True)

    with tile.TileContext(nc) as tc:
        with (
            tc.tile_pool(name="feat", bufs=1) as featp,
            tc.tile_pool(name="wpool", bufs=1) as wpool,
            tc.tile_pool(name="mpool", bufs=1) as mpool,
            tc.tile_pool(name="spool", bufs=4) as spool,
            tc.tile_pool(name="zpool", bufs=1) as zpool,
            tc.tile_pool(name="psum", bufs=1, space="PSUM") as psump,
        ):
            # ---- DMA plan: 3 HWDGE rings, bytes balanced, need-ordered ----
            # feat in 4 two-chunk tiles (long rows); W in 8-block groups.
            feats = [None] * NCHUNK
            wgs = {}

            def load_feat(cpair):
                ft = featp.tile([128, 2, BPC, FEATW], XW_DT,
                                name=f"feat{cpair}")
                feats[2 * cpair] = ft
                return (ft[:], xs_h[:, 2 * cpair:2 * cpair + 2])

            def load_wg(g):
                gcols = int(gw_off[g + 1] - gw_off[g])
                wg = wpool.tile([128, gcols], XW_DT, name=f"wg{g}")
                wgs[g] = wg
                return (wg[:], wf_h[:, int(gw_off[g]):int(gw_off[g + 1])])

            zAll = zpool.tile([128, 2 * nblk], F32)
            nc.vector.memset(zAll[:], 0.0)
            biasT_t = zpool.tile([128, 2 * nblk], F32)
            mask_t = mpool.tile([128, mtot], F32)

            plan = {
                nc.sync:   [load_feat(0), load_feat(2), load_wg(1)],
                nc.scalar: [load_feat(1), load_feat(3), load_wg(2),
                            (biasT_t[:], bt_h[:])],
                nc.gpsimd: [load_wg(0), (mask_t[:], mf_h[:]), load_wg(3)],
            }
            maxlen = max(len(v) for v in plan.values())
            for k in range(maxlen):
                for eng, items in plan.items():
                    if k < len(items):
                        dst, srcap = items[k]
                        eng.dma_start(dst, srcap)

            for g in range(ngrp):
                blks = list(range(g * GRP, min(nblk, (g + 1) * GRP)))
                wg = wgs[g]
                pms = {}
                for i in blks:
                    pms[i] = psump.tile([128, 2, wins[i]], F32,
                                        name=f"pm{i}", tag=f"pm{i % 8}")
                for c in range(NCHUNK):
                    for i in blks:
                        m, win, pfirst = ms[i], wins[i], pfirsts[i]
                        o = gcol_off[i]
                        nc.tensor.matmul(
                            pms[i][0:m, :, :],
                            wg[:, o + c * m:o + (c + 1) * m],
                            feats[2 * (c // 2)][:, c % 2, :,
                                                pfirst:pfirst + win],
                            start=(c == 0),
                            stop=(c == NCHUNK - 1),
                        )
                for i in blks:
                    m, win = ms[i], wins[i]
                    mo = int(moffs[i])
                    mk = mask_t[0:m, mo:mo + win].unsqueeze(1).broadcast_to(
                        (m, 2, win)
                    )
                    masked = spool.tile([128, 2, WINMAX], F32, tag="mx")
                    nc.vector.tensor_mul(
                        masked[0:m, :, 0:win], pms[i][0:m, :, :], mk
                    )
                    nc.vector.tensor_reduce(
                        zAll[0:m, 2 * i:2 * i + 2],
                        masked[0:m, :, 0:win],
                        axis=mybir.AxisListType.X,
                        op=mybir.AluOpType.add,
                    )

            # y = z + bias ; out = elu(y)+1 = exp(-relu(-y)) + relu(y)
            yt = zpool.tile([128, 2 * nblk], F32)
            nc.vector.tensor_add(yt[:], zAll[:], biasT_t[:])
            rp = zpool.tile([128, 2 * nblk], F32)
            nc.scalar.activation(rp[:], yt[:], mybir.ActivationFunctionType.Relu)
            rn = zpool.tile([128, 2 * nblk], F32)
            nc.scalar.activation(
                rn[:], yt[:], mybir.ActivationFunctionType.Relu, scale=-1.0
            )
            ep = zpool.tile([128, 2 * nblk], F32)
            nc.scalar.activation(
                ep[:], rn[:], mybir.ActivationFunctionType.Exp, scale=-1.0
            )
            ot = zpool.tile([128, 2 * nblk], F32)
            nc.vector.tensor_add(ot[:], ep[:], rp[:])
            nc.sync.dma_start(z_h[:], ot[:])

    _split_waits(nc)

    # ---- run on 8 cores ----
    # xs packed to the exact SBUF layout [128, chunk, batch, FEATW] (zero
    # padded), so each feat DMA moves one long contiguous row per partition.
    xr = x.reshape(B, NCHUNK, 128, P).astype(XW_NP)
    wall_np = wall.astype(XW_NP)
    in_maps = []
    for core in range(NCORES):
        xs_dev = np.zeros((128, NCHUNK, BPC, FEATW), XW_NP)
        xs_dev[:, :, :, :P] = (
            xr[BPC * core:BPC * (core + 1)].transpose(2, 1, 0, 3)
        )
        in_maps.append({
            "xs": xs_dev,
            "wf": wall_np,
            "mf": mask_all,
            "bt": biasT,
        })
    res = run_bass_kernel_spmd(nc, in_maps, core_ids=list(range(NCORES)))

    # ---- assemble ----
    y = np.empty((B, N), np.float32)
    for core in range(NCORES):
        z = res.results[core]["z"]
        for i, (s, e) in enumerate(blocks):
            idx = order[s:e]
            m = e - s
            y[BPC * core, idx] = z[0:m, 2 * i]
            y[BPC * core + 1, idx] = z[0:m, 2 * i + 1]
    return y


# revision 25
# speedup vs baseline: 1.1278x; 1.1278x over previous
"""PoissonGaussianReadout forward on 8 trn2 NeuronCores.

Math (eval mode): each neuron n samples feat[b] (a [36,36,1024] image per
batch, 1024 = C*T channels) bilinearly at a fixed point mu[n], then takes a
per-neuron dot with W[n,:], adds b[n], applies elu(y)+1.

Strategy:
  - Batch-shard: 8 cores x 2 batches each; every core computes all 4096
    neurons for its 2 batches (min HBM traffic: 10.6MB x-shard + 16.8MB W).
  - Sort neurons by their bilinear base cell p00 = y0*36+x0.  A block of
    <=128 sorted neurons spans a contiguous window of flat positions
    [pfirst, pfirst+WIN).  One fp32 matmul per (block, d-chunk):
       psum[n, (b,j)] += Wblk[d, n]^T @ feat[b, pfirst+j, d-chunk]
    i.e. the moving operand is a *contiguous slice* of feat -- no gather.
  - Each neuron's 4 bilinear corners live at window offsets
    (p00-pfirst)+{0,1,36,37}; a host-built sparse mask [n, b, j] holds the
    bilinear weights there.  DVE: masked = psum * mask; reduce_j -> z[n, b].
  - Bias + elu(y)+1 = exp(-relu(-y)) + relu(y) on device, output assembled
    and un-permuted on host.

The block structure / masks depend on mu, which is known when kernel() is
called; the Bass program is traced fresh per call, so they are baked in as
compile-time constants (correct for any input values).
"""
import sys
sys.path.insert(0, "/opt/trn_rl_repo")

import numpy as np

from concourse import bass, mybir, tile
from concourse.bass_utils import run_bass_kernel_spmd
from concourse.vector_clock import ScopedClock
import bass_rust

# problem constants
B, C, T, HH, WW = 16, 64, 16, 36, 36
N, D = 4096, C * T            # 4096 neurons, 1024 input dim
P = HH * WW                   # 1296 flat positions
NCHUNK = 8                    # D / 128 contraction chunks
NCORES = 8
BPC = B // NCORES             # batches per core = 2
WINMAX = 256                  # psum bank: 2*WIN <= 512 fp32
PAD = 38                      # max corner offset (37) + 1
FEATW = P + PAD               # padded feat width per (chunk, batch)

F32 = mybir.dt.float32

# dtype for x / W (the matmul operands).  bf16 halves DMA traffic and
# roughly halves PE time (FWL weight loads + 1cyc/col stream); PSUM
# accumulation stays fp32.  Set to False for bit-accurate fp32.
USE_BF16 = True
if USE_BF16:
    import ml_dtypes
    XW_DT = mybir.dt.bfloat16
    XW_NP = ml_dtypes.bfloat16
else:
    XW_DT = F32
    XW_NP = np.float32


def _split_waits(nc, max_waits=1):
    """Walrus in this image allows only ONE sem wait per instruction.
    Hoist extra waits onto injected same-engine NoOps placed immediately
    before the owning instruction (same engine + program order => same
    semantics)."""
    k = 0
    for fn in nc.m.functions:
        for blk in fn.blocks:
            insts = blk.instructions
            out = []
            for inst in insts:
                si = inst.sync_info
                if si is not None and si.on_wait and len(si.on_wait) > max_waits:
                    waits = list(si.on_wait)
                    for w in waits[:-max_waits]:
                        nop = mybir.InstNoOp(name=f"I-wsplit-{k}", ins=[], outs=[])
                        k += 1
                        nop.engine = inst.engine
                        nop.sync_info = bass_rust.SyncInfo(
                            on_wait=[w], on_update=[]
                        )
                        out.append(nop)
                    si.on_wait = waits[-max_waits:]
                    inst.sync_info = si
                out.append(inst)
            if len(out) != len(insts):
                insts.clear()
                insts.extend(out)


def _bilinear_tables(mu):
    """Per-neuron base cell p00, corner offsets (4) in {0,1,36,37}, corner
    weights (4), replicating reference float32 arithmetic exactly."""
    one, half = np.float32(1.0), np.float32(0.5)
    g = np.clip(mu.astype(np.float32), -one, one)
    ix = (g[:, 0] + one) * np.float32(WW * 0.5) - half
    iy = (g[:, 1] + one) * np.float32(HH * 0.5) - half
    x0 = np.floor(ix)
    y0 = np.floor(iy)
    wx1 = ix - x0
    wx0 = one - wx1
    wy1 = iy - y0
    wy0 = one - wy1

    xs = [x0, x0 + one]
    ys = [y0, y0 + one]
    wxs = [wx0, wx1]
    wys = [wy0, wy1]

    x0c = np.clip(x0, 0, WW - 1).astype(np.int64)
    y0c = np.clip(y0, 0, HH - 1).astype(np.int64)
    p00 = y0c * WW + x0c

    offs = np.zeros((4, N), np.int64)
    wgts = np.zeros((4, N), np.float32)
    k = 0
    for a in range(2):          # y corner
        for bb in range(2):     # x corner
            xx, yy = xs[bb], ys[a]
            valid = (xx >= 0) & (xx <= WW - 1) & (yy >= 0) & (yy <= HH - 1)
            xi = np.clip(xx, 0, WW - 1).astype(np.int64)
            yi = np.clip(yy, 0, HH - 1).astype(np.int64)
            offs[k] = yi * WW + xi - p00
            wgts[k] = (wys[a] * wxs[bb]) * valid.astype(np.float32)
            k += 1
    assert offs.min() >= 0 and offs.max() <= 37
    return p00, offs, wgts


def _make_blocks(p00_sorted):
    """Greedy blocks of <=128 sorted neurons with window <= WINMAX."""
    blocks = []  # (start, end) into sorted order
    s = 0
    n = len(p00_sorted)
    while s < n:
        pfirst = p00_sorted[s]
        e = s
        while e < n and e - s < 128 and (p00_sorted[e] - pfirst) + PAD <= WINMAX:
            e += 1
        blocks.append((s, e))
        s = e
    return blocks


def kernel(x, mu, sigma, W, b):
    x = np.ascontiguousarray(x, dtype=np.float32)
    W = np.ascontiguousarray(W, dtype=np.float32)
    b = np.asarray(b, dtype=np.float32)

    p00, offs, wgts = _bilinear_tables(mu)
    order = np.argsort(p00, kind="stable")
    p00s = p00[order]
    blocks = _make_blocks(p00s)
    nblk = len(blocks)

    # per-block host data
    wins, pfirsts, ms = [], [], []
    wparts, mparts = [], []
    biasT = np.zeros((128, 2 * nblk), np.float32)
    for i, (s, e) in enumerate(blocks):
        idx = order[s:e]
        m = e - s
        pfirst = int(p00s[s])
        win = int(p00s[e - 1]) - pfirst + PAD
        ms.append(m)
        pfirsts.append(pfirst)
        wins.append(win)
        # weights: [m,1024] -> [1024,m] -> [8,128,m] -> [128,8,m]
        blkW = W[idx, :].T.reshape(NCHUNK, 128, m).transpose(1, 0, 2)
        wparts.append(np.ascontiguousarray(blkW).reshape(128, NCHUNK * m))
        # mask [128, win] (b-dim broadcast on device)
        mk = np.zeros((128, win), np.float32)
        rel = (p00[idx] - pfirst)  # [m]
        for k in range(4):
            np.add.at(mk[:m], (np.arange(m), rel + offs[k][idx]), wgts[k][idx])
        mparts.append(mk)
        biasT[:m, 2 * i] = b[idx]
        biasT[:m, 2 * i + 1] = b[idx]

    # W packed per GRP-block group, partition-major: each group's DMA moves
    # [128, sum_i 8*m_i] with one long contiguous row per partition.
    GRP = 8
    ngrp = (nblk + GRP - 1) // GRP
    wgrps = []       # per-group [128, gcols] array
    gcol_off = []    # per-block column offset within its group (elements)
    for g in range(ngrp):
        gparts = []
        coff = 0
        for i in range(g * GRP, min(nblk, (g + 1) * GRP)):
            gcol_off.append(coff)
            gparts.append(wparts[i])          # [128, 8*m_i]
            coff += NCHUNK * ms[i]
        wgrps.append(np.ascontiguousarray(np.concatenate(gparts, axis=1)))
    wall = np.ascontiguousarray(np.concatenate(wgrps, axis=1))
    gw_off = np.cumsum([0] + [g.shape[1] for g in wgrps])  # col offset per group
    # all masks packed into one resident [128, sum(win)] tile
    mask_all = np.ascontiguousarray(np.concatenate(mparts, axis=1))
    moffs = np.cumsum([0] + [w for w in wins])
    mtot = int(mask_all.shape[1])

    # ---- build the Bass program (same for all cores) ----
    nc = bass.Bass()
    xs_h = nc.declare_dram_parameter("xs", [128, NCHUNK, BPC, FEATW], XW_DT,
                                     isOutput=False)
    wf_h = nc.declare_dram_parameter("wf", [128, int(wall.shape[1])], XW_DT,
                                     isOutput=False)
    mf_h = nc.declare_dram_parameter("mf", [128, mtot], F32, isOutput=False)
    bt_h = nc.declare_dram_parameter("bt", [128, 2 * nblk], F32, isOutput=False)
    z_h = nc.declare_dram_parameter("z", [128, 2 * nblk], F32, isOutput=True)

    with tile.TileContext(nc) as tc:
        with (
            tc.tile_pool(name="feat", bufs=1) as featp,
            tc.tile_pool(name="wpool", bufs=1) as wpool,
            tc.tile_pool(name="mpool", bufs=1) as mpool,
            tc.tile_pool(name="spool", bufs=4) as spool,
            tc.tile_pool(name="zpool", bufs=1) as zpool,
            tc.tile_pool(name="psum", bufs=1, space="PSUM") as psump,
        ):
            # ---- DMA plan: 3 HWDGE rings, bytes balanced, need-ordered ----
            # feat in 4 two-chunk tiles (long rows); W in 8-block groups.
            feats = [None] * NCHUNK
            wgs = {}

            def load_feat(cpair):
                ft = featp.tile([128, 2, BPC, FEATW], XW_DT,
                                name=f"feat{cpair}")
                feats[2 * cpair] = ft
                return (ft[:], xs_h[:, 2 * cpair:2 * cpair + 2])

            def load_wg(g):
                gcols = int(gw_off[g + 1] - gw_off[g])
                wg = wpool.tile([128, gcols], XW_DT, name=f"wg{g}")
                wgs[g] = wg
                return (wg[:], wf_h[:, int(gw_off[g]):int(gw_off[g + 1])])

            zAll = zpool.tile([128, 2 * nblk], F32)
            nc.vector.memset(zAll[:], 0.0)
            biasT_t = zpool.tile([128, 2 * nblk], F32)
            mask_t = mpool.tile([128, mtot], F32)

            plan = {
                nc.sync:   [load_wg(0), load_feat(0), load_feat(2),
                            load_wg(2)],
                nc.scalar: [load_feat(1), load_feat(3), load_wg(1),
                            load_wg(3)],
                nc.gpsimd: [(mask_t[:], mf_h[:]), (biasT_t[:], bt_h[:])],
            }
            maxlen = max(len(v) for v in plan.values())
            for k in range(maxlen):
                for eng, items in plan.items():
                    if k < len(items):
                        dst, srcap = items[k]
                        eng.dma_start(dst, srcap)

            for g in range(ngrp):
                blks = list(range(g * GRP, min(nblk, (g + 1) * GRP)))
                wg = wgs[g]
                pms = {}
                for i in blks:
                    pms[i] = psump.tile([128, 2, wins[i]], F32,
                                        name=f"pm{i}", tag=f"pm{i % 8}")
                for c in range(NCHUNK):
                    for i in blks:
                        m, win, pfirst = ms[i], wins[i], pfirsts[i]
                        o = gcol_off[i]
                        nc.tensor.matmul(
                            pms[i][0:m, :, :],
                            wg[:, o + c * m:o + (c + 1) * m],
                            feats[2 * (c // 2)][:, c % 2, :,
                                                pfirst:pfirst + win],
                            start=(c == 0),
                            stop=(c == NCHUNK - 1),
                        )
                for i in blks:
                    m, win = ms[i], wins[i]
                    mo = int(moffs[i])
                    mk = mask_t[0:m, mo:mo + win].unsqueeze(1).broadcast_to(
                        (m, 2, win)
                    )
                    masked = spool.tile([128, 2, WINMAX], F32, tag="mx")
                    nc.vector.tensor_mul(
                        masked[0:m, :, 0:win], pms[i][0:m, :, :], mk
                    )
                    nc.vector.tensor_reduce(
                        zAll[0:m, 2 * i:2 * i + 2],
                        masked[0:m, :, 0:win],
                        axis=mybir.AxisListType.X,
                        op=mybir.AluOpType.add,
                    )

                # per-group tail: y = z + bias ; out = exp(-relu(-y)) + relu(y)
                sl = slice(2 * blks[0], 2 * (blks[-1] + 1))
                gk = len(blks) * 2
                yt = spool.tile([128, 2 * GRP], F32, tag="yt")
                nc.vector.tensor_add(yt[:, 0:gk], zAll[:, sl], biasT_t[:, sl])
                rp = spool.tile([128, 2 * GRP], F32, tag="rp")
                nc.scalar.activation(rp[:, 0:gk], yt[:, 0:gk],
                                     mybir.ActivationFunctionType.Relu)
                rn = spool.tile([128, 2 * GRP], F32, tag="rn")
                nc.scalar.activation(rn[:, 0:gk], yt[:, 0:gk],
                                     mybir.ActivationFunctionType.Relu,
                                     scale=-1.0)
                ep = spool.tile([128, 2 * GRP], F32, tag="ep")
                nc.scalar.activation(ep[:, 0:gk], rn[:, 0:gk],
                                     mybir.ActivationFunctionType.Exp,
                                     scale=-1.0)
                ot = spool.tile([128, 2 * GRP], F32, tag="ot")
                nc.vector.tensor_add(ot[:, 0:gk], ep[:, 0:gk], rp[:, 0:gk])
                nc.gpsimd.dma_start(z_h[:, sl], ot[:, 0:gk])

    _split_waits(nc)

    # ---- run on 8 cores ----
    # xs packed to the exact SBUF layout [128, chunk, batch, FEATW] (zero
    # padded), so each feat DMA moves one long contiguous row per partition.
    xr = x.reshape(B, NCHUNK, 128, P).astype(XW_NP)
    wall_np = wall.astype(XW_NP)
    in_maps = []
    for core in range(NCORES):
        xs_dev = np.zeros((128, NCHUNK, BPC, FEATW), XW_NP)
        xs_dev[:, :, :, :P] = (
            xr[BPC * core:BPC * (core + 1)].transpose(2, 1, 0, 3)
        )
        in_maps.append({
            "xs": xs_dev,
            "wf": wall_np,
            "mf": mask_all,
            "bt": biasT,
        })
    res = run_bass_kernel_spmd(nc, in_maps, core_ids=list(range(NCORES)))

    # ---- assemble ----
    y = np.empty((B, N), np.float32)
    for core in range(NCORES):
        z = res.results[core]["z"]
        for i, (s, e) in enumerate(blocks):
            idx = order[s:e]
            m = e - s
            y[BPC * core, idx] = z[0:m, 2 * i]
            y[BPC * core + 1, idx] = z[0:m, 2 * i + 1]
    return y
